# revision 1
# baseline (speedup 1.0000x reference)
# Trainium2 Bass kernel for nn_MultiHeadAttention_85933705658435
#
# Reference: LayerNorm(scale-only) -> QKV projection -> full softmax attention
#            -> output projection.  x:[S,B,E] f32, S=2048 B=2 E=1024, H=16 D=64.
#
# Sharding (8 cores): batch (2-way) x head-group (4-way, 4 heads/core).
#   - Each core LayerNorms its batch's 2048 tokens (redundant across the 4
#     cores of a batch - cheaper than a collective at this size).
#   - W_qkv column-sharded (the core's 4 heads), W_out row-sharded; the
#     4 partial output projections per batch are summed on the host
#     (gather/unshard step of the row-sharded matmul).
#
# Layout strategy on-core: everything "feature-major" (transposed) so that the
# attention pipeline needs no on-chip transposes of big intermediates:
#   ln^T [E, S] (via DRAM-bounce DMA transpose, bf16)
#   Q^T,K^T,V^T [D,S] per head -> scores^T [k,q] = (K^T chunk).T @ Q^T  (K=64,
#       two heads packed in the PE array via 2x row tiling)
#   V bounced through DRAM to token-major V' with an appended ones column:
#       the ctx matmul (M=65) then yields both ctx^T and the softmax
#       denominators (no separate reduction).
#   exp on ACT psum->sbuf (bf16), no max-subtraction (scores ~ N(0,1) by
#       construction: the reference scales the query block by 1/sqrt(D));
#       the attention k-loop is software-pipelined (ctx trails scores by one
#       chunk) so the PE stays busy and the HAM clock stays at 2.4 GHz.
#   out-proj consumes ctx^T directly as the stationary operand.

import numpy as np
import ml_dtypes

S, B, E = 2048, 2, 1024
H, D = 16, 64
HPC = 4              # heads per core
NCORES = 8
EPS = 1e-6
FQK = HPC * D        # 256 (per-core Q width = K width = V width)
P = 128
TC = S // P          # 16 token chunks
ECH = E // P         # 8 e-chunks
QTS = 1024           # q-tile size in attention
NQT = S // QTS       # 2

BF16 = ml_dtypes.bfloat16

_CACHE = {}


def _build_nc():
    from contextlib import ExitStack

    import concourse.bass as bass
    import concourse.tile as tile
    from concourse import bacc, mybir
    from concourse.tile import add_dep_helper

    dt = mybir.dt
    Alu = mybir.AluOpType
    Act = mybir.ActivationFunctionType

    nc = bacc.Bacc(trn_type="TRN2")
    x_d = nc.dram_tensor("x", (S, E), dt.float32, kind="ExternalInput").ap()
    # wqkv: [E, 3*FQK] = Q | K | V column blocks for this core's 4 heads
    wqkv_d = nc.dram_tensor(
        "wqkv", (E, 3 * FQK), dt.bfloat16, kind="ExternalInput"
    ).ap()
    wo_d = nc.dram_tensor("wo", (FQK, E), dt.bfloat16, kind="ExternalInput").ap()
    out_d = nc.dram_tensor("out", (S, E), dt.float32, kind="ExternalOutput").ap()

    NFC = 6  # f-chunks of 128: 0,1=Q  2,3=K  4,5=V

    with tile.TileContext(nc) as tc, ExitStack() as ctx:
        singles = ctx.enter_context(tc.tile_pool(name="singles", bufs=1))
        xp = ctx.enter_context(tc.tile_pool(name="xp", bufs=3))
        lnp = ctx.enter_context(tc.tile_pool(name="lnp", bufs=3))
        small = ctx.enter_context(tc.tile_pool(name="small", bufs=4))
        expp = ctx.enter_context(tc.tile_pool(name="expp", bufs=3))
        evac = ctx.enter_context(tc.tile_pool(name="evac", bufs=2))
        dram = ctx.enter_context(tc.tile_pool(name="dram", bufs=1, space="DRAM"))

        # persistent SBUF tensors
        lnT = singles.tile([P, ECH, S], dt.bfloat16)          # ln^T, e-chunked
        qkT = singles.tile([P, 4, S], dt.bfloat16)            # fc 0,1: Q^T; 2,3: K^T
        Vp = singles.tile([P, TC, HPC * (D + 1)], dt.bfloat16)  # token-major V + ones
        vt_sb = singles.tile([P, 2, S], dt.bfloat16)          # V^T staging
        vt2_sb = singles.tile([P, TC, FQK], dt.bfloat16)      # V t-major staging
        w_sb = singles.tile([P, ECH, 3 * FQK], dt.bfloat16)
        wo_sb = singles.tile([P, 2, E], dt.bfloat16)
        ones1 = singles.tile([1, D], dt.bfloat16)
        eps_sb = singles.tile([P, 1], dt.float32)
        ctxn = singles.tile([P, 2, S], dt.bfloat16)           # normalized ctx^T / pair
        ln_dram = dram.tile([S, E], dt.bfloat16)
        vt_dram = dram.tile([FQK, S], dt.bfloat16)
        rc_dram = dram.tile([8, QTS], dt.float32)

        nc.vector.memset(eps_sb[:], EPS)
        nc.sync.dma_start(w_sb[:], wqkv_d.rearrange("(c p) f -> p c f", p=P))
        nc.sync.dma_start(wo_sb[:], wo_d.rearrange("(c p) e -> p c e", p=P))
        nc.vector.memset(ones1[:], 1.0)
        # ones columns of V' (position D within each head's 65-wide block)
        nc.vector.memset(
            Vp.rearrange("p t (h z) -> p t h z", z=D + 1)[:, :, :, D : D + 1], 1.0
        )

        # ---- Phase 1: LayerNorm (token-major), bf16 out, bounce via DRAM ----
        ln_writes = []
        for t in range(TC):
            xb = xp.tile([P, E], dt.float32, tag="xb")
            # x loads on the Activation hwdge queue: plain copies are safe
            # there (only dma_start_transpose on that queue corrupts), and it
            # takes 8.4MB off the sync queue that also carries the ln bounce
            nc.scalar.dma_start(xb[:], x_d[t * P : (t + 1) * P, :])
            st = small.tile([P, 2, 6], dt.float32, tag="st")
            nc.vector.bn_stats(st[:, 0, :], xb[:, 0:512])
            nc.vector.bn_stats(st[:, 1, :], xb[:, 512:1024])
            mv = small.tile([P, 2], dt.float32, tag="mv")
            nc.vector.bn_aggr(mv[:], st[:])
            sd = small.tile([P, 1], dt.float32, tag="sd")
            nc.scalar.activation(sd[:], mv[:, 1:2], Act.Sqrt, bias=eps_sb[:])
            rs = small.tile([P, 1], dt.float32, tag="rs")
            nc.vector.reciprocal(rs[:], sd[:])
            lnb = lnp.tile([P, E], dt.bfloat16, tag="lnb")
            if t % 2 == 0:
                nc.vector.tensor_scalar(
                    lnb[:], xb[:], mv[:, 0:1], rs[:], Alu.subtract, Alu.mult
                )
            else:
                # offload half the normalizes to the otherwise-idle ACT:
                # ln = x*rs + (-mu*rs)
                nb = small.tile([P, 1], dt.float32, tag="nb")
                nc.vector.tensor_tensor(nb[:], mv[:, 0:1], rs[:], Alu.mult)
                nc.vector.tensor_scalar_mul(nb[:], nb[:], -1.0)
                nc.scalar.activation(
                    lnb[:], xb[:], Act.Identity, bias=nb[:], scale=rs[:]
                )
            ln_writes.append(
                nc.sync.dma_start(ln_dram[t * P : (t + 1) * P, :], lnb[:])
            )

        # ---- Phase 2: DMA-transpose ln -> ln^T (split over 2 queues) ----
        # DRAM pool tiles are not dependency-tracked by the Tile framework
        # (MANAGED_SPACES is SBUF/PSUM only), so the RAW edges through
        # ln_dram must be added explicitly.
        for c in range(ECH):
            tp = nc.sync.dma_start_transpose(
                lnT[:, c, :], ln_dram[:, c * P : (c + 1) * P]
            )
            for wi in ln_writes:
                add_dep_helper(tp.ins, wi.ins, True, "lnT RAW via ln_dram")

        # ---- Phase 3: QKV^T feature-major, weights-stationary ----
        with tc.tile_pool(name="psA", bufs=2, space="PSUM") as psA:
            for fc in (4, 5, 0, 2, 1, 3):  # V first so its DRAM bounce overlaps
                pstt = [
                    psA.tile([P, 512], dt.float32, tag=f"qk{tt}", name=f"ps{tt}")
                    for tt in range(4)
                ]
                for ec in range(ECH):
                    for tt in range(4):
                        nc.tensor.matmul(
                            pstt[tt][:],
                            w_sb[:, ec, fc * P : (fc + 1) * P],
                            lnT[:, ec, tt * 512 : (tt + 1) * 512],
                            start=(ec == 0),
                            stop=(ec == ECH - 1),
                        )
                for tt in range(4):
                    dst = (
                        qkT[:, fc, tt * 512 : (tt + 1) * 512]
                        if fc < 4
                        else vt_sb[:, fc - 4, tt * 512 : (tt + 1) * 512]
                    )
                    nc.vector.tensor_copy(dst, pstt[tt][:])
            # keep the PE streaming through the psA->psB pool transition so
            # the HAM clock stays at 8/8 entering the attention phase
            wps0 = psA.tile([P, 512], dt.float32, tag="qk0", name="wps0")
            for _ in range(18):
                nc.tensor.matmul(
                    wps0[:], w_sb[:, 0, 0:P], lnT[:, 0, 0:512],
                    start=True, stop=True,
                )

        # ---- Phase 4: V^T -> token-major V' via DRAM bounce ----
        vt_writes = []
        for half in range(2):
            vt_writes.append(
                nc.sync.dma_start(
                    vt_dram[half * P : (half + 1) * P, :], vt_sb[:, half, :]
                )
            )
        for t in range(TC):
            tp = nc.sync.dma_start_transpose(
                vt2_sb[:, t, :], vt_dram[:, t * P : (t + 1) * P]
            )
            for wi in vt_writes:
                add_dep_helper(tp.ins, wi.ins, True, "V RAW via vt_dram")
            nc.vector.tensor_copy(
                Vp[:, t, :].rearrange("p (h z) -> p h z", z=D + 1)[:, :, 0:D],
                vt2_sb[:, t, :].rearrange("p (h z) -> p h z", z=D),
            )

        # ---- Phase 5: attention (2 head pairs x 2 q-tiles x 16 k-chunks) ----
        # software-pipelined: ctx MMs for chunk kc-1 are emitted after the
        # scores MMs for chunk kc, so the PE never idles long enough to lose
        # the HAM 2.4 GHz clock while ACT computes exp.
        with tc.tile_pool(name="psB", bufs=1, space="PSUM") as psB:
            def warmup(n_mm):
                # back-to-back throwaway matmuls to push the PE HAM clock
                # gate to 8/8 (2.4 GHz) before a dependency-coupled phase
                wps = psB.tile([P, 512], dt.float32, tag="sB", name="wps")
                for _ in range(n_mm):
                    nc.tensor.matmul(
                        wps[:], qkT[:, 0, 0:P], qkT[:, 0, 0:512],
                        start=True, stop=True,
                    )

            for pr in range(2):        # head pair (local heads 2pr, 2pr+1)
                for qt in range(NQT):
                    warmup(10)
                    q0 = qt * QTS
                    cps = [
                        psB.tile(
                            [D + 1, QTS], dt.float32, tag=f"ctx{h}", name=f"cps{h}"
                        )
                        for h in range(2)
                    ]

                    def scores(kc):
                        k0 = kc * P
                        sA = psB.tile([P, QTS], dt.float32, tag="sA", name="sA")
                        sB = psB.tile([P, QTS], dt.float32, tag="sB", name="sB")
                        for n in range(QTS // 512):
                            nc.tensor.matmul(
                                sA[:, n * 512 : (n + 1) * 512],
                                qkT[0:64, 2 + pr, k0 : k0 + P],
                                qkT[0:64, pr, q0 + n * 512 : q0 + (n + 1) * 512],
                                start=True,
                                stop=True,
                                tile_position=(0, 0),
                            )
                        for n in range(QTS // 512):
                            nc.tensor.matmul(
                                sB[:, n * 512 : (n + 1) * 512],
                                qkT[64:128, 2 + pr, k0 : k0 + P],
                                qkT[64:128, pr, q0 + n * 512 : q0 + (n + 1) * 512],
                                start=True,
                                stop=True,
                                tile_position=(64, 0),
                            )
                        eA = expp.tile([P, QTS], dt.bfloat16, tag="eA", name="eA")
                        eB = expp.tile([P, QTS], dt.bfloat16, tag="eB", name="eB")
                        nc.scalar.activation(eA[:], sA[:], Act.Exp)
                        nc.scalar.activation(eB[:], sB[:], Act.Exp)
                        return eA, eB

                    def ctx_mms(kc, exps):
                        for h, ex in ((0, exps[0]), (1, exps[1])):
                            hh = pr * 2 + h
                            for n in range(QTS // 512):
                                nc.tensor.matmul(
                                    cps[h][:, n * 512 : (n + 1) * 512],
                                    Vp[:, kc, hh * (D + 1) : (hh + 1) * (D + 1)],
                                    ex[:, n * 512 : (n + 1) * 512],
                                    start=(kc == 0),
                                    stop=(kc == TC - 1),
                                )

                    prev = scores(0)
                    for kc in range(1, TC):
                        cur = scores(kc)
                        ctx_mms(kc - 1, prev)
                        prev = cur
                    ctx_mms(TC - 1, prev)

                    # normalize: ctx^T[d,q] / denom[q]; denom is ctx row D.
                    # Evacuate the ctx psums to SBUF first (frees the psum
                    # banks so the next block's attention proceeds), then do
                    # the whole normalize chain in SBUF off the critical path:
                    #   - reciprocal of a [1,1024] row is ~6us on one DVE
                    #     lane, so reshape to [128,8] via tiny SBUF DMAs
                    #   - broadcast the recip row across 64 partitions with a
                    #     step-0 DMA (no PE/psum involved)
                    for h in range(2):
                        ctxu = evac.tile([D + 1, QTS], dt.float32, tag=f"ctxu{h}",
                                         name=f"ctxu{h}")
                        nc.vector.tensor_copy(ctxu[:], cps[h][:, :])
                        dnp = small.tile([P, QTS // P], dt.float32, tag="dnp")
                        nc.sync.dma_start(dnp[:], ctxu[D : D + 1, :])
                        rcp = small.tile([P, QTS // P], dt.float32, tag="rcp")
                        nc.vector.reciprocal(rcp[:], dnp[:])
                        slot = (pr * NQT + qt) * 2 + h
                        rc_row = rc_dram[slot : slot + 1, :]
                        wr = nc.sync.dma_start(rc_row, rcp[:])
                        bcs = evac.tile([64, QTS], dt.float32, tag="bcs")
                        rc_bcast = bass.AP(
                            tensor=rc_row.tensor,
                            offset=rc_row.offset,
                            ap=[[0, 64]] + list(rc_row.ap[1:]),
                        )
                        rd = nc.sync.dma_start(bcs[:], rc_bcast)
                        add_dep_helper(rd.ins, wr.ins, True, "recip RAW via dram")
                        if h == 0:
                            nc.vector.tensor_tensor(
                                ctxn[0:64, pr, q0 : q0 + QTS],
                                ctxu[0:D, :],
                                bcs[:],
                                Alu.mult,
                            )
                        else:
                            tmpn = evac.tile([64, QTS], dt.bfloat16, tag="tmpn")
                            nc.vector.tensor_tensor(
                                tmpn[:], ctxu[0:D, :], bcs[:], Alu.mult
                            )
                            # partition shift 0-63 -> 64-127 via SBUF-SBUF DMA
                            nc.sync.dma_start(ctxn[64:128, pr, q0 : q0 + QTS], tmpn[:])

        # ---- Phase 6: output projection ----
        with tc.tile_pool(name="psC", bufs=3, space="PSUM") as psC:
            # brief warm burst bridging the psB->psC pool transition
            wpc = psC.tile([P, E], dt.float32, tag="op", name="wpc")
            for _ in range(8):
                nc.tensor.matmul(
                    wpc[:, 0:512], ctxn[:, 0, 0:P], wo_sb[:, 0, 0:512],
                    start=True, stop=True,
                )
            for t in range(TC):
                po = psC.tile([P, E], dt.float32, tag="op")
                for pr in range(2):
                    for et in range(2):
                        nc.tensor.matmul(
                            po[:, et * 512 : (et + 1) * 512],
                            ctxn[:, pr, t * P : (t + 1) * P],
                            wo_sb[:, pr, et * 512 : (et + 1) * 512],
                            start=(pr == 0),
                            stop=(pr == 1),
                        )
                ob = evac.tile([P, E], dt.float32, tag="ob", bufs=3)
                for et in range(2):
                    nc.any.tensor_copy(
                        ob[:, et * 512 : (et + 1) * 512],
                        po[:, et * 512 : (et + 1) * 512],
                    )
                    nc.sync.dma_start(
                        out_d[t * P : (t + 1) * P, et * 512 : (et + 1) * 512],
                        ob[:, et * 512 : (et + 1) * 512],
                    )

    nc.compile()
    return nc


def make_in_maps(x, ln_scale, w_qkv, w_out):
    w = (np.asarray(w_qkv, np.float32) * np.asarray(ln_scale, np.float32)[:, None])
    wo = np.asarray(w_out, np.float32)
    in_maps = []
    for c in range(NCORES):
        b, g = divmod(c, 4)
        h0 = g * HPC
        wq = w[:, h0 * D : (h0 + HPC) * D]
        wk = w[:, H * D + h0 * D : H * D + (h0 + HPC) * D]
        wv = w[:, 2 * H * D + h0 * D : 2 * H * D + (h0 + HPC) * D]
        in_maps.append(
            {
                "x": np.ascontiguousarray(np.asarray(x, np.float32)[:, b, :]),
                "wqkv": np.ascontiguousarray(
                    np.concatenate([wq, wk, wv], axis=1)
                ).astype(BF16),
                "wo": np.ascontiguousarray(
                    wo[h0 * D : (h0 + HPC) * D, :]
                ).astype(BF16),
            }
        )
    return in_maps


def get_nc():
    if "nc" not in _CACHE:
        _CACHE["nc"] = _build_nc()
    return _CACHE["nc"]


def assemble(results):
    out = np.empty((S, B, E), np.float32)
    for b in range(B):
        acc = results[4 * b]["out"].astype(np.float32).copy()
        for g in range(1, 4):
            acc += results[4 * b + g]["out"]
        out[:, b, :] = acc
    return out


def kernel(x, ln_scale, w_qkv, w_out):
    from concourse.bass_utils import run_bass_kernel_spmd

    nc = get_nc()
    in_maps = make_in_maps(x, ln_scale, w_qkv, w_out)
    res = run_bass_kernel_spmd(nc, in_maps, core_ids=list(range(NCORES)))
    return assemble(res.results)

